# revision 64
# baseline (speedup 1.0000x reference)
"""GAT layer kernel for Trainium2, 8 NeuronCores, data-parallel over R=b*s.

Self-contained: takes full inputs, returns full output.

v4.6 design (per core, RC=6 replicas):
  - Phase A: projection on PE in bf16 (lhsT padded to 128 for FWL);
    psum->SBUF copies split ACT/DVE into two per-half row-image slabs
    [125, 4, 1664] (h c-major per r + a_src bf16 + pad).  The h table
    lives in HBM as two 500-row halves; each is written by one Pool-queue
    dma_start right when its nodes are projected, so the src<500 gathers
    start while phase A still projects nodes 500-999.
  - Self-loop edges never enter the gather: per-dst-tile "self tile"
    multiplies the slab directly, feeding num/den through a static
    identity one-hot (self opens the accumulation groups).
  - Non-self edges sorted by (dst-tile, src-half); 16 direct SWDGE
    gathers on the Pool queue (inline descgen ~5.5us stays ahead of the
    ~7.4us/sub data stream).  Queue FIFO orders h-writes before gathers;
    gather k is gated on consumption of the hg buffer it recycles
    (5 buffers) via an explicit PE sem_inc after the last num matmul.
    Per-sub DMA-completion sems; all Pool waits/gathers chained with
    no-sync deps so Tile cannot reorder them, and the consumer-side
    data-readiness waits are pinned to the first hg reader per engine.
  - z = a_src[src]+a_dst[dst] built in PSUM by PE (ohT @ ad accumulated
    with I @ gathered-as) in a dedicated 1-bank PSUM tile (z + den_lo +
    den_hi) so PSUM reads never overlap open groups in the 3-bank num
    tile; leaky-relu via ACT copy + DVE stt; exp on ACT.
  - Post-norm pipeline: per sub-chunk, right after exp: den one-shot
    group, msg = p * hg in place (DVE 2x, c-major broadcast), num
    accumulation (3x512-col matmuls, oh fp8 padded to 128 cols for FWL).
  - Finalize per dst-tile: invden = 0.25/(den_lo+den_hi) (head-mean
    folded), numn = num * invden, head-sums at 2x via (h0+h2, h1+h3)
    pairing, + bias, per-dst-tile fat store.
"""

import math
import numpy as np
import ml_dtypes

B, S, N, F = 4, 12, 1000, 64
H, C = 4, 64
HC = H * C            # 256
R = B * S             # 48
NCORES = 8
RC = R // NCORES      # 6 replicas per core
NEG_SLOPE = 0.2
DTW = 125             # dst-tile width (8 tiles cover N=1000)
NDT = N // DTW        # 8
AC = RC * H           # 24 active scalar columns
ROWW = 1664           # h_hbm row width in bf16 (6*256 h + 24 as + pad) = 3328B
NSUB = 2 * NDT        # 16 gather sub-chunks (dst-tile x src-half)

_CACHE = {}

F8 = ml_dtypes.float8_e4m3fn
BF16 = ml_dtypes.bfloat16


# --------------------------------------------------------------------------
# host-side index preprocessing
# --------------------------------------------------------------------------
def _prep_edges(edge_index):
    src0 = np.asarray(edge_index[0], dtype=np.int64)
    dst0 = np.asarray(edge_index[1], dtype=np.int64)
    keep = src0 != dst0          # PyG remove_self_loops (masked with NEG_INF)
    s_all, d_all = src0[keep], dst0[keep]

    subs = []                    # per sub-chunk: src list, dst-local list
    for dt in range(NDT):
        m = (d_all >= dt * DTW) & (d_all < (dt + 1) * DTW)
        ss, dd = s_all[m], d_all[m] - dt * DTW
        order = np.argsort(ss, kind="stable")
        ss, dd = ss[order], dd[order]
        for half in range(2):
            hm = (ss >= half * 500) & (ss < (half + 1) * 500)
            sh, dh = ss[hm], dd[hm]
            cnt = len(sh)
            nt = max(1, math.ceil(cnt / 128))
            pad = nt * 128 - cnt
            # pad slots read a real row of the half-table (weight 0 in oh)
            sh = np.concatenate([sh, np.full(pad, half * 500, np.int64)])
            dh = np.concatenate([dh, np.full(pad, -1, np.int64)])  # no dst
            subs.append(dict(dt=dt, half=half, nt=nt, src=sh, dl=dh))

    T = sum(s["nt"] for s in subs)      # total gathered slot-tiles
    nt_dt = [subs[2 * dt]["nt"] + subs[2 * dt + 1]["nt"] for dt in range(NDT)]

    oh = np.zeros((128, T, 128), F8)    # num/den lhsT (slots x dst-local)
    ohT = np.zeros((128, T, 128), F8)   # z/invden expand lhsT (dst x slots)
    tot_idx = T * 128
    ihw = np.zeros((128, tot_idx // 16), np.int16)
    t0 = 0
    sub_meta = []
    for sb in subs:
        nt, ss, dd = sb["nt"], sb["src"], sb["dl"]
        ni = nt * 128
        for j in range(ni):
            if dd[j] >= 0:
                p, t = j % 128, j // 128
                oh[p, t0 + t, dd[j]] = 1.0
                ohT[dd[j], t0 + t, p] = 1.0
        a = np.zeros((16, ni // 16), np.int16)
        idx = np.arange(ni)
        # indices are local to the half's sliced h table
        a[idx % 16, idx // 16] = (ss - sb["half"] * 500).astype(np.int16)
        ihw[:, t0 * 8:(t0 + nt) * 8] = np.tile(a, (8, 1))
        sub_meta.append(dict(dt=sb["dt"], half=sb["half"], nt=nt, t0=t0))
        t0 += nt

    selfI = np.zeros((128, 128), F8)    # self-tile lhsT: I at [d, d], d<125
    for d in range(DTW):
        selfI[d, d] = 1.0
    I128 = np.zeros((128, 128), F8)     # as-add lhsT (full identity)
    for p in range(128):
        I128[p, p] = 1.0

    return {
        "T": T, "subs": sub_meta, "nt_dt": nt_dt,
        "oh": np.ascontiguousarray(oh.reshape(128, T * 128)),
        "ohT": np.ascontiguousarray(ohT.reshape(128, T * 128)),
        "ih": ihw, "selfI": selfI, "I128": I128,
    }


def _prep_weights(W, att_src, att_dst):
    W = np.asarray(W, np.float32)
    Ws = np.zeros((F, H), np.float32)
    Wd = np.zeros((F, H), np.float32)
    for h in range(H):
        Ws[:, h] = W[:, h * C:(h + 1) * C] @ np.asarray(att_src, np.float32)[h]
        Wd[:, h] = W[:, h * C:(h + 1) * C] @ np.asarray(att_dst, np.float32)[h]
    # c-major head interleave: device col c*4+h = W col h*64+c
    Wc = np.empty_like(W)
    for h in range(H):
        Wc[:, np.arange(C) * H + h] = W[:, h * C:(h + 1) * C]
    # head-mean 1/H is folded into invden on-device, NOT here
    waug = np.concatenate([Wc, Ws, Wd], axis=1)             # [64, 264]
    return waug.astype(BF16)


def _make_in_maps(x, W, att_src, att_dst, bias, ed):
    waug = _prep_weights(W, att_src, att_dst)
    bias_slab = np.tile(np.asarray(bias, np.float32)[None, None, :],
                        (DTW, RC, 1))                        # [125, 6, 64]
    xr = np.ascontiguousarray(np.asarray(x, np.float32)).reshape(R, N, F)
    in_maps = []
    for cidx in range(NCORES):
        xc = xr[cidx * RC:(cidx + 1) * RC]
        xT = np.ascontiguousarray(xc.transpose(2, 0, 1).reshape(F, RC * N)
                                  ).astype(BF16)
        in_maps.append({
            "xT": xT, "w_aug": waug, "oh": ed["oh"], "ohT": ed["ohT"],
            "ih": ed["ih"], "selfI": ed["selfI"], "I128": ed["I128"],
            "bias_slab": np.ascontiguousarray(bias_slab).reshape(DTW, RC * F),
        })
    return in_maps


# --------------------------------------------------------------------------
# device program
# --------------------------------------------------------------------------
def _build_program(ed):
    import concourse.bass as bass
    import concourse.mybir as mybir
    from concourse.instruction_name_ordered_set import InstructionNameOrderedSet as INOS
    import concourse.tile as tile
    from concourse import bacc

    T = ed["T"]
    subs = ed["subs"]
    nt_dt = ed["nt_dt"]
    f32 = mybir.dt.float32
    bf16 = mybir.dt.bfloat16
    fp8 = mybir.dt.float8e4
    i16 = mybir.dt.int16
    Alu = mybir.AluOpType
    Act = mybir.ActivationFunctionType

    nc = bacc.Bacc("TRN2", target_bir_lowering=False, debug=False,
                   enable_asserts=False, num_devices=NCORES)

    xT_d = nc.dram_tensor("xT", [F, RC * N], bf16, kind="ExternalInput").ap()
    waug_d = nc.dram_tensor("w_aug", [F, 264], bf16, kind="ExternalInput").ap()
    oh_d = nc.dram_tensor("oh", [128, T * 128], fp8, kind="ExternalInput").ap()
    ohT_d = nc.dram_tensor("ohT", [128, T * 128], fp8, kind="ExternalInput").ap()
    ih_d = nc.dram_tensor("ih", [128, T * 8], i16, kind="ExternalInput").ap()
    selfI_d = nc.dram_tensor("selfI", [128, 128], fp8, kind="ExternalInput").ap()
    I128_d = nc.dram_tensor("I128", [128, 128], fp8, kind="ExternalInput").ap()
    bias_d = nc.dram_tensor("bias_slab", [DTW, RC * F], f32,
                            kind="ExternalInput").ap()
    out_d = nc.dram_tensor("out", [RC, N, F], f32, kind="ExternalOutput").ap()

    MAXSNT = max(s["nt"] for s in subs)
    MAXNT = max(nt_dt)
    HGBUFS = 5

    # z/den live in their own 1-bank PSUM tile so PSUM reads of one tile
    # never overlap another tile's open accumulation groups:
    ZC0 = 0                       # z region: cols 0 .. nt*24
    DEN0 = MAXNT * AC             # den_lo / den_hi: 2x24 cols

    with tile.TileContext(nc) as tc:
        with (
            tc.tile_pool(name="const", bufs=1) as constp,
            tc.tile_pool(name="dram", bufs=1, space="DRAM") as dramp,
            tc.tile_pool(name="hgp", bufs=HGBUFS) as hgp,
            tc.tile_pool(name="edge", bufs=3) as edgep,
            tc.tile_pool(name="fin", bufs=2) as finp,
        ):
            h_lo = dramp.tile([500, ROWW], bf16)
            h_hi = dramp.tile([500, ROWW], bf16)

            # ---- constant loads (sync + scalar queues; Pool stays clean) --
            waug = constp.tile([F, 264], bf16)
            nc.sync.dma_start(waug[:], waug_d)
            ih = constp.tile([128, T * 8], i16)
            nc.scalar.dma_start(ih[:], ih_d)
            oh = constp.tile([128, T, 128], fp8)
            nc.scalar.dma_start(oh[:], oh_d.rearrange("p (t e) -> p t e", e=128))
            ohT = constp.tile([128, T, 128], fp8)
            nc.scalar.dma_start(ohT[:], ohT_d.rearrange("p (t e) -> p t e", e=128))
            selfI = constp.tile([128, 128], fp8)
            nc.scalar.dma_start(selfI[:], selfI_d)
            I128 = constp.tile([128, 128], fp8)
            nc.scalar.dma_start(I128[:], I128_d)
            bias_sl = constp.tile([DTW, RC, F], f32)
            nc.scalar.dma_start(bias_sl[:], bias_d.rearrange("p (r f) -> p r f", f=F))

            # projection slabs: row image per node, split by half so the
            # lo gathers never falsely depend on the hi writes
            slab_lo = constp.tile([DTW, 4, ROWW], bf16)
            slab_hi = constp.tile([DTW, 4, ROWW], bf16)
            adsl = constp.tile([DTW, NDT, RC, 8], bf16)   # as+ad staging

            # slab pad columns are DMA'd but never written by phase A
            nc.vector.memset(slab_lo[:, :, RC * HC + AC:], 0.0)
            nc.vector.memset(slab_hi[:, :, RC * HC + AC:], 0.0)

            def slab_at(a):
                return (slab_lo, a) if a < 4 else (slab_hi, a - 4)

            gsems = [nc.alloc_semaphore(f"gsem{k}") for k in range(NSUB)]

            def gwait(eng, k):
                return eng.wait_ge(gsems[k], 16)
            csem = nc.alloc_semaphore("csem")       # hg tile consumed

            hg_tiles = []
            for _ in subs:
                hgt = hgp.tile([128, MAXSNT, ROWW], bf16, tag="hg")
                hg_tiles.append(hgt)

            # dummy gather: loads the Pool gather ucode library during
            # phase A so the first real gather pays no LIBRARY_RELOAD
            row0 = constp.tile([1, 128], bf16)
            nc.vector.memset(row0[:], 0.0)
            nc.scalar.dma_start(h_lo[0:1, 0:128], row0[:])
            ih0 = constp.tile([128, 1], i16)
            nc.vector.memset(ih0[:], 0)
            dummy_hg = constp.tile([128, 1, 128], bf16)
            dummy_g = nc.gpsimd.dma_gather(
                out_ap=dummy_hg[:], in_ap=h_lo[:, 0:128], idxs_ap=ih0[:],
                num_idxs=16, num_idxs_reg=16, elem_size=128,
                elem_step=ROWW, single_packet=False)

            # ---- phase A: projection; fills slab/adsl, writes h_hbm ------
            with (
                tc.tile_pool(name="stage", bufs=1) as stagep,
                tc.tile_pool(name="ppsum", bufs=6, space="PSUM") as ppsum,
            ):
                xts = []
                for r in range(RC):
                    xt = stagep.tile([F, N], bf16, tag=f"xt{r}")
                    nc.sync.dma_start(xt[:], xT_d[:, r * N:(r + 1) * N])
                    xts.append(xt)
                for a in range(NDT):
                    sl, al = slab_at(a)
                    for r in range(RC):
                        n0 = a * DTW
                        kw = 128 if n0 + 128 <= N else DTW   # FWL when 128
                        ps = ppsum.tile([128, 264], f32, tag="proj")
                        nc.tensor.matmul(out=ps[0:kw, :],
                                         lhsT=xts[r][:, n0:n0 + kw],
                                         rhs=waug[:], start=True, stop=True)
                        if r < 2:
                            nc.scalar.copy(
                                out=sl[:, al, r * HC:(r + 1) * HC],
                                in_=ps[0:DTW, 0:HC])
                        else:
                            nc.vector.tensor_copy(
                                out=sl[:, al, r * HC:(r + 1) * HC],
                                in_=ps[0:DTW, 0:HC])
                        nc.scalar.copy(out=adsl[:, a, r, :],
                                       in_=ps[0:DTW, HC:HC + 8])
                    # a_src into the row image (r-contiguous bf16 cols)
                    nc.vector.tensor_copy(
                        out=sl[:, al, RC * HC:RC * HC + AC].rearrange(
                            "d (r e) -> d r e", e=H),
                        in_=adsl[:, a, :, 0:H])
                    if a == 3:     # lo half of h table written (nodes < 500)
                        hlo_w = nc.sync.dma_start(
                            h_lo[:].rearrange("(a d) w -> d a w", d=DTW),
                            slab_lo[:])
                hfull_w = nc.sync.dma_start(
                    h_hi[:].rearrange("(a d) w -> d a w", d=DTW),
                    slab_hi[:])

                # self-loop attention logits for ALL nodes in one shot:
                # z_self = as + ad ; leaky ; exp -> p_self [125, 8, 24]
                zs = stagep.tile([DTW, NDT, RC, H], bf16, tag="zs")
                nc.vector.tensor_tensor(out=zs[:], in0=adsl[:, :, :, 0:H],
                                        in1=adsl[:, :, :, 4:8], op=Alu.add)
                dve_anchor = nc.vector.scalar_tensor_tensor(
                    out=zs[:], in0=zs[:], scalar=NEG_SLOPE, in1=zs[:],
                    op0=Alu.mult, op1=Alu.max)
                p_self = constp.tile([DTW, NDT, AC], bf16)
                pe_anchor = nc.scalar.activation(
                    out=p_self[:].rearrange("d a (r h) -> d a r h", h=H),
                    in_=zs[:], func=Act.Exp)

            # ---- Pool: direct gathers (descgen inline; ~3us each stays
            # ahead of the ~7.4us/sub DMA stream).  Gather k gated on the
            # h-half write and on consumption of the tile it recycles; all
            # waits and gathers chained with no-sync deps so Tile cannot
            # reorder them on the Pool stream.
            # Tile wires gather->h-write RAW deps automatically (reads of
            # h_lo/h_hi vs the sync-queue writes)
            prev_g = dummy_g
            for k, sb in enumerate(subs):
                deps = [prev_g.ins.name]
                if k >= HGBUFS:
                    # hg buffer ring rotates in SUB order: sub k reuses the
                    # buffer of sub k-HGBUFS; csem counts consumptions in
                    # sub order too
                    w = nc.gpsimd.wait_ge(csem, k - HGBUFS + 1)
                    w.ins.add_nosync_dependencies_from(INOS([prev_g.ins.name]))
                    deps.append(w.ins.name)
                snt = sb["nt"]
                ni = snt * 128
                t0 = sb["t0"]
                gi = nc.gpsimd.dma_gather(
                    out_ap=hg_tiles[k][:, 0:snt, :],
                    in_ap=(h_lo[:] if sb["half"] == 0 else h_hi[:]),
                    idxs_ap=ih[:, t0 * 8:(t0 + snt) * 8],
                    num_idxs=ni, num_idxs_reg=ni, elem_size=ROWW,
                    single_packet=False)
                gi.then_inc(gsems[k], 16)
                gi.ins.add_nosync_dependencies_from(INOS(deps))
                prev_g = gi

            # ---- edge phase: per dst-tile --------------------------------
            pending_fin = [None]
            with tc.tile_pool(name="npsum", bufs=2, space="PSUM") as npsum:
                for dt in range(NDT):
                    klo, khi = 2 * dt, 2 * dt + 1
                    slo, shi = subs[klo], subs[khi]
                    ntd = nt_dt[dt]
                    nps = npsum.tile([128, 1536], f32, tag="num")
                    zdn = npsum.tile([128, 512], f32, tag="zdn")

                    p_sb = edgep.tile([128, MAXNT, AC], bf16, tag="p")

                    # explicit data-readiness waits for the deferred gather
                    # writes: pinned to the first hg reader per engine via
                    # no-sync deps (raw waits float under Tile scheduling).
                    # self-loop messages (unnormalized; only needs phase A)
                    msgs = finp.tile([DTW, RC * HC], bf16, tag="msgs")
                    nc.vector.tensor_tensor(
                        out=msgs[:].rearrange("d (r c h) -> d r c h", h=H, c=C),
                        in0=slab_at(dt)[0][:, slab_at(dt)[1],
                                          0:RC * HC].rearrange(
                            "d (r c h) -> d r c h", h=H, c=C),
                        in1=p_self[:, dt, :].rearrange(
                            "d (r o h) -> d r o h", h=H, o=1
                        ).to_broadcast([DTW, RC, C, H]),
                        op=Alu.mult)
                    # num group opens with the self tile (banks 0-2)
                    for cb in range(3):
                        nc.tensor.matmul(
                            out=nps[:, cb * 512:(cb + 1) * 512],
                            lhsT=selfI[0:DTW, :],
                            rhs=msgs[:, cb * 512:(cb + 1) * 512],
                            start=True, stop=False)

                    # per sub-chunk: z (PE), leaky+exp, den, msg, num
                    for k, sb in ((klo, slo), (khi, shi)):
                        snt = sb["nt"]
                        t0 = sb["t0"]
                        toff = 0 if k == klo else slo["nt"]
                        hg = hg_tiles[k]
                        pe_w = gwait(nc.tensor, k)
                        pe_w.ins.add_nosync_dependencies_from(
                            INOS([pe_anchor.ins.name]))
                        pe_anchor = pe_w
                        dve_w = gwait(nc.vector, k)
                        dve_w.ins.add_nosync_dependencies_from(
                            INOS([dve_anchor.ins.name]))
                        dve_anchor = dve_w
                        first_pe = True
                        for t in range(snt):
                            zc = ZC0 + (toff + t) * AC
                            nc.tensor.matmul(
                                out=zdn[:, zc:zc + AC],
                                lhsT=ohT[0:DTW, t0 + t, :],
                                rhs=adsl[:, dt, :, 4:8],
                                start=True, stop=False)
                            im = nc.tensor.matmul(
                                out=zdn[:, zc:zc + AC],
                                lhsT=I128[:],
                                rhs=hg[:, t, RC * HC:RC * HC + AC],
                                start=False, stop=True)
                            if first_pe:
                                im.ins.add_nosync_dependencies_from(
                                    INOS([pe_w.ins.name]))
                                first_pe = False
                        zc0 = ZC0 + toff * AC
                        zsb = edgep.tile([128, MAXSNT, AC], bf16, tag="zsb")
                        nc.scalar.copy(
                            out=zsb[:, 0:snt, :],
                            in_=zdn[:, zc0:zc0 + snt * AC].rearrange(
                                "p (t a) -> p t a", a=AC))
                        nc.vector.scalar_tensor_tensor(
                            out=p_sb[:, toff:toff + snt, :],
                            in0=zsb[:, 0:snt, :], scalar=NEG_SLOPE,
                            in1=zsb[:, 0:snt, :], op0=Alu.mult, op1=Alu.max)
                        nc.scalar.activation(
                            out=p_sb[:, toff:toff + snt, :],
                            in_=p_sb[:, toff:toff + snt, :], func=Act.Exp)

                        # den for this half, own one-shot group (self rides
                        # in the lo group so bank-3 groups never interleave)
                        dc = DEN0 if k == klo else DEN0 + AC
                        if k == klo:
                            nc.tensor.matmul(out=zdn[:, dc:dc + AC],
                                             lhsT=selfI[0:DTW, :],
                                             rhs=p_self[:, dt, :],
                                             start=True, stop=False)
                        for t in range(snt):
                            nc.tensor.matmul(out=zdn[:, dc:dc + AC],
                                             lhsT=oh[:, t0 + t, :],
                                             rhs=p_sb[:, toff + t, :],
                                             start=(k == khi and t == 0),
                                             stop=(t == snt - 1))

                        # msg = p * hg in place, then num accumulation
                        hgv = hg[:, 0:snt, 0:RC * HC].rearrange(
                            "p t (r c h) -> p t r c h", h=H, c=C)
                        pb = p_sb[:, toff:toff + snt, :].rearrange(
                            "p t (r o h) -> p t r o h", h=H, o=1
                        ).to_broadcast([128, snt, RC, C, H])
                        mv = nc.vector.tensor_tensor(out=hgv, in0=hgv, in1=pb,
                                                     op=Alu.mult)
                        mv.ins.add_nosync_dependencies_from(
                            INOS([dve_w.ins.name]))
                        last = None
                        for t in range(snt):
                            for cb in range(3):
                                last = nc.tensor.matmul(
                                    out=nps[:, cb * 512:(cb + 1) * 512],
                                    lhsT=oh[:, t0 + t, :],
                                    rhs=hg[:, t, cb * 512:(cb + 1) * 512],
                                    start=False,
                                    stop=(k == khi and t == snt - 1))
                        ci = nc.tensor.sem_inc(csem, 1)
                        ci.ins.add_nosync_dependencies_from(
                            INOS([last.ins.name]))
                        # run the PREVIOUS dt's finalize after this dt's
                        # first sub so the DVE doesn't block on PE num(dt-1)
                        # while fresh gather data waits
                        if k == klo and pending_fin[0] is not None:
                            pending_fin[0]()
                            pending_fin[0] = None

                    def make_fin(dt=dt, nps=nps, zdn=zdn):
                        def fin():
                            # finalize: numn = num*(0.25/den), head-sum, +bias
                            dpair = finp.tile([DTW, 2, AC], f32, tag="dpair")
                            nc.scalar.copy(
                                out=dpair[:],
                                in_=zdn[0:DTW,
                                        DEN0:DEN0 + 2 * AC].rearrange(
                                    "d (e a) -> d e a", a=AC))
                            dsum = finp.tile([DTW, AC], f32, tag="dsum")
                            nc.vector.tensor_tensor(
                                out=dsum[:], in0=dpair[:, 0, :],
                                in1=dpair[:, 1, :], op=Alu.add)
                            invd = finp.tile([DTW, AC], f32, tag="invd")
                            nc.vector.reciprocal(out=invd[:], in_=dsum[:])
                            invdb = finp.tile([DTW, AC], bf16, tag="invdb")
                            nc.vector.tensor_scalar_mul(invdb[:], invd[:],
                                                        0.25)
                            nsb = finp.tile([DTW, RC, C, H], bf16, tag="nsb")
                            nc.scalar.copy(
                                out=nsb[:],
                                in_=nps[0:DTW, 0:RC * HC].rearrange(
                                    "d (r c h) -> d r c h", h=H, c=C))
                            nc.vector.tensor_tensor(
                                out=nsb[:], in0=nsb[:],
                                in1=invdb[:].rearrange(
                                    "d (r o h) -> d r o h", h=H, o=1
                                ).to_broadcast([DTW, RC, C, H]),
                                op=Alu.mult)
                            hp = finp.tile([DTW, RC, C, 2], bf16, tag="hp")
                            nc.vector.tensor_tensor(out=hp[:],
                                                    in0=nsb[:, :, :, 0:2],
                                                    in1=nsb[:, :, :, 2:4],
                                                    op=Alu.add)
                            ob = finp.tile([DTW, RC, C], f32, tag="ob")
                            nc.vector.tensor_tensor(out=ob[:],
                                                    in0=hp[:, :, :, 0],
                                                    in1=hp[:, :, :, 1],
                                                    op=Alu.add)
                            nc.vector.tensor_tensor(out=ob[:], in0=ob[:],
                                                    in1=bias_sl[:],
                                                    op=Alu.add)
                            nc.sync.dma_start(
                                out_d[:, dt * DTW:(dt + 1) * DTW, :].rearrange(
                                    "r d f -> d r f"), ob[:])
                        return fin
                    pending_fin[0] = make_fin()
                if pending_fin[0] is not None:
                    pending_fin[0]()
                    pending_fin[0] = None

    nc.compile()
    return nc


# --------------------------------------------------------------------------
# public entry point
# --------------------------------------------------------------------------
def kernel(x, edge_index, W, att_src, att_dst, bias):
    key = hash(np.asarray(edge_index).tobytes())
    if key not in _CACHE:
        ed = _prep_edges(edge_index)
        _CACHE[key] = (_build_program(ed), ed)
    nc, ed = _CACHE[key]

    in_maps = _make_in_maps(x, W, att_src, att_dst, bias, ed)
    from concourse import bass_utils
    res = bass_utils.run_bass_kernel_spmd(nc, in_maps, core_ids=list(range(NCORES)))
    outs = [res.results[c]["out"] for c in range(NCORES)]
    out = np.concatenate(outs, axis=0).reshape(B, S, N, F).astype(np.float32)
    return out


# revision 65
# speedup vs baseline: 1.0122x; 1.0122x over previous
"""GAT layer kernel for Trainium2, 8 NeuronCores, data-parallel over R=b*s.

Self-contained: takes full inputs, returns full output.

v4.6 design (per core, RC=6 replicas):
  - Phase A: projection on PE in bf16 (lhsT padded to 128 for FWL);
    psum->SBUF copies split ACT/DVE into two per-half row-image slabs
    [125, 4, 1664] (h c-major per r + a_src bf16 + pad).  The h table
    lives in HBM as two 500-row halves; each is written by one Pool-queue
    dma_start right when its nodes are projected, so the src<500 gathers
    start while phase A still projects nodes 500-999.
  - Self-loop edges never enter the gather: per-dst-tile "self tile"
    multiplies the slab directly, feeding num/den through a static
    identity one-hot (self opens the accumulation groups).
  - Non-self edges sorted by (dst-tile, src-half); 16 direct SWDGE
    gathers on the Pool queue (inline descgen ~5.5us stays ahead of the
    ~7.4us/sub data stream).  Queue FIFO orders h-writes before gathers;
    gather k is gated on consumption of the hg buffer it recycles
    (5 buffers) via an explicit PE sem_inc after the last num matmul.
    Per-sub DMA-completion sems; all Pool waits/gathers chained with
    no-sync deps so Tile cannot reorder them, and the consumer-side
    data-readiness waits are pinned to the first hg reader per engine.
  - z = a_src[src]+a_dst[dst] built in PSUM by PE (ohT @ ad accumulated
    with I @ gathered-as) in a dedicated 1-bank PSUM tile (z + den_lo +
    den_hi) so PSUM reads never overlap open groups in the 3-bank num
    tile; leaky-relu via ACT copy + DVE stt; exp on ACT.
  - Post-norm pipeline: per sub-chunk, right after exp: den one-shot
    group, msg = p * hg in place (DVE 2x, c-major broadcast), num
    accumulation (3x512-col matmuls, oh fp8 padded to 128 cols for FWL).
  - Finalize per dst-tile: invden = 0.25/(den_lo+den_hi) (head-mean
    folded), numn = num * invden, head-sums at 2x via (h0+h2, h1+h3)
    pairing, + bias, per-dst-tile fat store.
"""

import math
import numpy as np
import ml_dtypes

B, S, N, F = 4, 12, 1000, 64
H, C = 4, 64
HC = H * C            # 256
R = B * S             # 48
NCORES = 8
RC = R // NCORES      # 6 replicas per core
NEG_SLOPE = 0.2
DTW = 125             # dst-tile width (8 tiles cover N=1000)
NDT = N // DTW        # 8
AC = RC * H           # 24 active scalar columns
ROWW = 1664           # h_hbm row width in bf16 (6*256 h + 24 as + pad) = 3328B
NSUB = 2 * NDT        # 16 gather sub-chunks (dst-tile x src-half)

_CACHE = {}

F8 = ml_dtypes.float8_e4m3fn
BF16 = ml_dtypes.bfloat16


# --------------------------------------------------------------------------
# host-side index preprocessing
# --------------------------------------------------------------------------
def _prep_edges(edge_index):
    src0 = np.asarray(edge_index[0], dtype=np.int64)
    dst0 = np.asarray(edge_index[1], dtype=np.int64)
    keep = src0 != dst0          # PyG remove_self_loops (masked with NEG_INF)
    s_all, d_all = src0[keep], dst0[keep]

    subs = []                    # per sub-chunk: src list, dst-local list
    for dt in range(NDT):
        m = (d_all >= dt * DTW) & (d_all < (dt + 1) * DTW)
        ss, dd = s_all[m], d_all[m] - dt * DTW
        order = np.argsort(ss, kind="stable")
        ss, dd = ss[order], dd[order]
        for half in range(2):
            hm = (ss >= half * 500) & (ss < (half + 1) * 500)
            sh, dh = ss[hm], dd[hm]
            cnt = len(sh)
            nt = max(1, math.ceil(cnt / 128))
            pad = nt * 128 - cnt
            # pad slots read a real row of the half-table (weight 0 in oh)
            sh = np.concatenate([sh, np.full(pad, half * 500, np.int64)])
            dh = np.concatenate([dh, np.full(pad, -1, np.int64)])  # no dst
            subs.append(dict(dt=dt, half=half, nt=nt, src=sh, dl=dh))

    T = sum(s["nt"] for s in subs)      # total gathered slot-tiles
    nt_dt = [subs[2 * dt]["nt"] + subs[2 * dt + 1]["nt"] for dt in range(NDT)]

    oh = np.zeros((128, T, 128), F8)    # num/den lhsT (slots x dst-local)
    ohT = np.zeros((128, T, 128), F8)   # z/invden expand lhsT (dst x slots)
    tot_idx = T * 128
    ihw = np.zeros((128, tot_idx // 16), np.int16)
    t0 = 0
    sub_meta = []
    for sb in subs:
        nt, ss, dd = sb["nt"], sb["src"], sb["dl"]
        ni = nt * 128
        for j in range(ni):
            if dd[j] >= 0:
                p, t = j % 128, j // 128
                oh[p, t0 + t, dd[j]] = 1.0
                ohT[dd[j], t0 + t, p] = 1.0
        a = np.zeros((16, ni // 16), np.int16)
        idx = np.arange(ni)
        # indices are local to the half's sliced h table
        a[idx % 16, idx // 16] = (ss - sb["half"] * 500).astype(np.int16)
        ihw[:, t0 * 8:(t0 + nt) * 8] = np.tile(a, (8, 1))
        sub_meta.append(dict(dt=sb["dt"], half=sb["half"], nt=nt, t0=t0))
        t0 += nt

    selfI = np.zeros((128, 128), F8)    # self-tile lhsT: I at [d, d], d<125
    for d in range(DTW):
        selfI[d, d] = 1.0
    I128 = np.zeros((128, 128), F8)     # as-add lhsT (full identity)
    for p in range(128):
        I128[p, p] = 1.0

    return {
        "T": T, "subs": sub_meta, "nt_dt": nt_dt,
        "oh": np.ascontiguousarray(oh.reshape(128, T * 128)),
        "ohT": np.ascontiguousarray(ohT.reshape(128, T * 128)),
        "ih": ihw, "selfI": selfI, "I128": I128,
    }


def _prep_weights(W, att_src, att_dst):
    W = np.asarray(W, np.float32)
    Ws = np.zeros((F, H), np.float32)
    Wd = np.zeros((F, H), np.float32)
    for h in range(H):
        Ws[:, h] = W[:, h * C:(h + 1) * C] @ np.asarray(att_src, np.float32)[h]
        Wd[:, h] = W[:, h * C:(h + 1) * C] @ np.asarray(att_dst, np.float32)[h]
    # c-major head interleave: device col c*4+h = W col h*64+c
    Wc = np.empty_like(W)
    for h in range(H):
        Wc[:, np.arange(C) * H + h] = W[:, h * C:(h + 1) * C]
    # head-mean 1/H is folded into invden on-device, NOT here
    waug = np.concatenate([Wc, Ws, Wd], axis=1)             # [64, 264]
    return waug.astype(BF16)


def _make_in_maps(x, W, att_src, att_dst, bias, ed):
    waug = _prep_weights(W, att_src, att_dst)
    bias_slab = np.tile(np.asarray(bias, np.float32)[None, None, :],
                        (DTW, RC, 1))                        # [125, 6, 64]
    xr = np.ascontiguousarray(np.asarray(x, np.float32)).reshape(R, N, F)
    in_maps = []
    for cidx in range(NCORES):
        xc = xr[cidx * RC:(cidx + 1) * RC]
        xT = np.ascontiguousarray(xc.transpose(2, 0, 1).reshape(F, RC * N)
                                  ).astype(BF16)
        in_maps.append({
            "xT": xT, "w_aug": waug, "oh": ed["oh"], "ohT": ed["ohT"],
            "ih": ed["ih"], "selfI": ed["selfI"], "I128": ed["I128"],
            "bias_slab": np.ascontiguousarray(bias_slab).reshape(DTW, RC * F),
        })
    return in_maps


# --------------------------------------------------------------------------
# device program
# --------------------------------------------------------------------------
def _build_program(ed):
    import concourse.bass as bass
    import concourse.mybir as mybir
    from concourse.instruction_name_ordered_set import InstructionNameOrderedSet as INOS
    import concourse.tile as tile
    from concourse import bacc

    T = ed["T"]
    subs = ed["subs"]
    nt_dt = ed["nt_dt"]
    f32 = mybir.dt.float32
    bf16 = mybir.dt.bfloat16
    fp8 = mybir.dt.float8e4
    i16 = mybir.dt.int16
    Alu = mybir.AluOpType
    Act = mybir.ActivationFunctionType

    nc = bacc.Bacc("TRN2", target_bir_lowering=False, debug=False,
                   enable_asserts=False, num_devices=NCORES)

    xT_d = nc.dram_tensor("xT", [F, RC * N], bf16, kind="ExternalInput").ap()
    waug_d = nc.dram_tensor("w_aug", [F, 264], bf16, kind="ExternalInput").ap()
    oh_d = nc.dram_tensor("oh", [128, T * 128], fp8, kind="ExternalInput").ap()
    ohT_d = nc.dram_tensor("ohT", [128, T * 128], fp8, kind="ExternalInput").ap()
    ih_d = nc.dram_tensor("ih", [128, T * 8], i16, kind="ExternalInput").ap()
    selfI_d = nc.dram_tensor("selfI", [128, 128], fp8, kind="ExternalInput").ap()
    I128_d = nc.dram_tensor("I128", [128, 128], fp8, kind="ExternalInput").ap()
    bias_d = nc.dram_tensor("bias_slab", [DTW, RC * F], f32,
                            kind="ExternalInput").ap()
    out_d = nc.dram_tensor("out", [RC, N, F], f32, kind="ExternalOutput").ap()

    MAXSNT = max(s["nt"] for s in subs)
    MAXNT = max(nt_dt)
    HGBUFS = 5

    # z/den live in their own 1-bank PSUM tile so PSUM reads of one tile
    # never overlap another tile's open accumulation groups:
    ZC0 = 0                       # z region: cols 0 .. nt*24
    DEN0 = MAXNT * AC             # den_lo / den_hi: 2x24 cols

    with tile.TileContext(nc) as tc:
        with (
            tc.tile_pool(name="const", bufs=1) as constp,
            tc.tile_pool(name="dram", bufs=1, space="DRAM") as dramp,
            tc.tile_pool(name="hgp", bufs=HGBUFS) as hgp,
            tc.tile_pool(name="edge", bufs=3) as edgep,
            tc.tile_pool(name="fin", bufs=2) as finp,
        ):
            h_lo = dramp.tile([500, ROWW], bf16)
            h_hi = dramp.tile([500, ROWW], bf16)

            # ---- constant loads (sync + scalar queues; Pool stays clean) --
            waug = constp.tile([F, 264], bf16)
            nc.sync.dma_start(waug[:], waug_d)
            ih = constp.tile([128, T * 8], i16)
            nc.scalar.dma_start(ih[:], ih_d)
            oh = constp.tile([128, T, 128], fp8)
            nc.scalar.dma_start(oh[:], oh_d.rearrange("p (t e) -> p t e", e=128))
            ohT = constp.tile([128, T, 128], fp8)
            nc.scalar.dma_start(ohT[:], ohT_d.rearrange("p (t e) -> p t e", e=128))
            selfI = constp.tile([128, 128], fp8)
            nc.scalar.dma_start(selfI[:], selfI_d)
            I128 = constp.tile([128, 128], fp8)
            nc.scalar.dma_start(I128[:], I128_d)
            bias_sl = constp.tile([DTW, RC, F], f32)
            nc.scalar.dma_start(bias_sl[:], bias_d.rearrange("p (r f) -> p r f", f=F))

            # projection slabs: row image per node, split by half so the
            # lo gathers never falsely depend on the hi writes
            slab_lo = constp.tile([DTW, 4, ROWW], bf16)
            slab_hi = constp.tile([DTW, 4, ROWW], bf16)
            adsl = constp.tile([DTW, NDT, RC, 8], bf16)   # as+ad staging

            # slab pad columns are DMA'd but never written by phase A
            nc.vector.memset(slab_lo[:, :, RC * HC + AC:], 0.0)
            nc.vector.memset(slab_hi[:, :, RC * HC + AC:], 0.0)

            def slab_at(a):
                return (slab_lo, a) if a < 4 else (slab_hi, a - 4)

            gsems = [nc.alloc_semaphore(f"gsem{k}") for k in range(NSUB)]

            def gwait(eng, k):
                return eng.wait_ge(gsems[k], 16)
            csem = nc.alloc_semaphore("csem")       # hg tile consumed

            hg_tiles = []
            for _ in subs:
                hgt = hgp.tile([128, MAXSNT, ROWW], bf16, tag="hg")
                hg_tiles.append(hgt)

            # dummy gather: loads the Pool gather ucode library during
            # phase A so the first real gather pays no LIBRARY_RELOAD
            row0 = constp.tile([1, 128], bf16)
            nc.vector.memset(row0[:], 0.0)
            nc.scalar.dma_start(h_lo[0:1, 0:128], row0[:])
            ih0 = constp.tile([128, 1], i16)
            nc.vector.memset(ih0[:], 0)
            dummy_hg = constp.tile([128, 1, 128], bf16)
            dummy_g = nc.gpsimd.dma_gather(
                out_ap=dummy_hg[:], in_ap=h_lo[:, 0:128], idxs_ap=ih0[:],
                num_idxs=16, num_idxs_reg=16, elem_size=128,
                elem_step=ROWW, single_packet=False)

            # ---- phase A: projection; fills slab/adsl, writes h_hbm ------
            with (
                tc.tile_pool(name="stage", bufs=1) as stagep,
                tc.tile_pool(name="ppsum", bufs=6, space="PSUM") as ppsum,
            ):
                xts = []
                for r in range(RC):
                    xt = stagep.tile([F, N], bf16, tag=f"xt{r}")
                    nc.sync.dma_start(xt[:], xT_d[:, r * N:(r + 1) * N])
                    xts.append(xt)
                for a in range(NDT):
                    sl, al = slab_at(a)
                    for r in range(RC):
                        n0 = a * DTW
                        kw = 128 if n0 + 128 <= N else DTW   # FWL when 128
                        ps = ppsum.tile([128, 264], f32, tag="proj")
                        nc.tensor.matmul(out=ps[0:kw, :],
                                         lhsT=xts[r][:, n0:n0 + kw],
                                         rhs=waug[:], start=True, stop=True)
                        if r < 2:
                            nc.scalar.copy(
                                out=sl[:, al, r * HC:(r + 1) * HC],
                                in_=ps[0:DTW, 0:HC])
                        else:
                            nc.vector.tensor_copy(
                                out=sl[:, al, r * HC:(r + 1) * HC],
                                in_=ps[0:DTW, 0:HC])
                        nc.scalar.copy(out=adsl[:, a, r, :],
                                       in_=ps[0:DTW, HC:HC + 8])
                    # a_src into the row image (r-contiguous bf16 cols)
                    nc.vector.tensor_copy(
                        out=sl[:, al, RC * HC:RC * HC + AC].rearrange(
                            "d (r e) -> d r e", e=H),
                        in_=adsl[:, a, :, 0:H])
                    if a == 3:     # lo half of h table written (nodes < 500)
                        hlo_w = nc.sync.dma_start(
                            h_lo[:].rearrange("(a d) w -> d a w", d=DTW),
                            slab_lo[:])
                hfull_w = nc.sync.dma_start(
                    h_hi[:].rearrange("(a d) w -> d a w", d=DTW),
                    slab_hi[:])

                # self-loop attention logits for ALL nodes in one shot:
                # z_self = as + ad ; leaky ; exp -> p_self [125, 8, 24]
                zs = stagep.tile([DTW, NDT, RC, H], bf16, tag="zs")
                nc.vector.tensor_tensor(out=zs[:], in0=adsl[:, :, :, 0:H],
                                        in1=adsl[:, :, :, 4:8], op=Alu.add)
                dve_anchor = nc.vector.scalar_tensor_tensor(
                    out=zs[:], in0=zs[:], scalar=NEG_SLOPE, in1=zs[:],
                    op0=Alu.mult, op1=Alu.max)
                p_self = constp.tile([DTW, NDT, AC], bf16)
                pe_anchor = nc.scalar.activation(
                    out=p_self[:].rearrange("d a (r h) -> d a r h", h=H),
                    in_=zs[:], func=Act.Exp)

            # ---- Pool: direct gathers (descgen inline; ~3us each stays
            # ahead of the ~7.4us/sub DMA stream).  Gather k gated on the
            # h-half write and on consumption of the tile it recycles; all
            # waits and gathers chained with no-sync deps so Tile cannot
            # reorder them on the Pool stream.
            # Tile wires gather->h-write RAW deps automatically (reads of
            # h_lo/h_hi vs the sync-queue writes)
            prev_g = dummy_g
            for k, sb in enumerate(subs):
                deps = [prev_g.ins.name]
                if k >= HGBUFS:
                    # hg buffer ring rotates in SUB order: sub k reuses the
                    # buffer of sub k-HGBUFS; csem counts consumptions in
                    # sub order too
                    w = nc.gpsimd.wait_ge(csem, k - HGBUFS + 1)
                    w.ins.add_nosync_dependencies_from(INOS([prev_g.ins.name]))
                    deps.append(w.ins.name)
                snt = sb["nt"]
                ni = snt * 128
                t0 = sb["t0"]
                gi = nc.gpsimd.dma_gather(
                    out_ap=hg_tiles[k][:, 0:snt, :],
                    in_ap=(h_lo[:] if sb["half"] == 0 else h_hi[:]),
                    idxs_ap=ih[:, t0 * 8:(t0 + snt) * 8],
                    num_idxs=ni, num_idxs_reg=ni, elem_size=ROWW,
                    single_packet=False)
                gi.then_inc(gsems[k], 16)
                gi.ins.add_nosync_dependencies_from(INOS(deps))
                prev_g = gi

            # ---- edge phase: per dst-tile --------------------------------
            with tc.tile_pool(name="npsum", bufs=2, space="PSUM") as npsum:
                for dt in range(NDT):
                    klo, khi = 2 * dt, 2 * dt + 1
                    slo, shi = subs[klo], subs[khi]
                    ntd = nt_dt[dt]
                    nps = npsum.tile([128, 1536], f32, tag="num")
                    zdn = npsum.tile([128, 512], f32, tag="zdn")

                    p_sb = edgep.tile([128, MAXNT, AC], bf16, tag="p")

                    # explicit data-readiness waits for the deferred gather
                    # writes: pinned to the first hg reader per engine via
                    # no-sync deps (raw waits float under Tile scheduling).
                    # self-loop messages (unnormalized; only needs phase A)
                    msgs = finp.tile([DTW, RC * HC], bf16, tag="msgs")
                    nc.vector.tensor_tensor(
                        out=msgs[:].rearrange("d (r c h) -> d r c h", h=H, c=C),
                        in0=slab_at(dt)[0][:, slab_at(dt)[1],
                                          0:RC * HC].rearrange(
                            "d (r c h) -> d r c h", h=H, c=C),
                        in1=p_self[:, dt, :].rearrange(
                            "d (r o h) -> d r o h", h=H, o=1
                        ).to_broadcast([DTW, RC, C, H]),
                        op=Alu.mult)
                    # num group opens with the self tile (banks 0-2)
                    for cb in range(3):
                        nc.tensor.matmul(
                            out=nps[:, cb * 512:(cb + 1) * 512],
                            lhsT=selfI[0:DTW, :],
                            rhs=msgs[:, cb * 512:(cb + 1) * 512],
                            start=True, stop=False)

                    # per sub-chunk: z (PE), leaky+exp, den, msg, num
                    for k, sb in ((klo, slo), (khi, shi)):
                        snt = sb["nt"]
                        t0 = sb["t0"]
                        toff = 0 if k == klo else slo["nt"]
                        hg = hg_tiles[k]
                        pe_w = gwait(nc.tensor, k)
                        pe_w.ins.add_nosync_dependencies_from(
                            INOS([pe_anchor.ins.name]))
                        pe_anchor = pe_w
                        dve_w = gwait(nc.vector, k)
                        dve_w.ins.add_nosync_dependencies_from(
                            INOS([dve_anchor.ins.name]))
                        dve_anchor = dve_w
                        first_pe = True
                        for t in range(snt):
                            zc = ZC0 + (toff + t) * AC
                            nc.tensor.matmul(
                                out=zdn[:, zc:zc + AC],
                                lhsT=ohT[0:DTW, t0 + t, :],
                                rhs=adsl[:, dt, :, 4:8],
                                start=True, stop=False)
                            im = nc.tensor.matmul(
                                out=zdn[:, zc:zc + AC],
                                lhsT=I128[:],
                                rhs=hg[:, t, RC * HC:RC * HC + AC],
                                start=False, stop=True)
                            if first_pe:
                                im.ins.add_nosync_dependencies_from(
                                    INOS([pe_w.ins.name]))
                                first_pe = False
                        zc0 = ZC0 + toff * AC
                        zsb = edgep.tile([128, MAXSNT, AC], bf16, tag="zsb")
                        nc.scalar.copy(
                            out=zsb[:, 0:snt, :],
                            in_=zdn[:, zc0:zc0 + snt * AC].rearrange(
                                "p (t a) -> p t a", a=AC))
                        nc.vector.scalar_tensor_tensor(
                            out=p_sb[:, toff:toff + snt, :],
                            in0=zsb[:, 0:snt, :], scalar=NEG_SLOPE,
                            in1=zsb[:, 0:snt, :], op0=Alu.mult, op1=Alu.max)
                        nc.scalar.activation(
                            out=p_sb[:, toff:toff + snt, :],
                            in_=p_sb[:, toff:toff + snt, :], func=Act.Exp)

                        # den for this half, own one-shot group (self rides
                        # in the lo group so bank-3 groups never interleave)
                        dc = DEN0 if k == klo else DEN0 + AC
                        if k == klo:
                            nc.tensor.matmul(out=zdn[:, dc:dc + AC],
                                             lhsT=selfI[0:DTW, :],
                                             rhs=p_self[:, dt, :],
                                             start=True, stop=False)
                        for t in range(snt):
                            nc.tensor.matmul(out=zdn[:, dc:dc + AC],
                                             lhsT=oh[:, t0 + t, :],
                                             rhs=p_sb[:, toff + t, :],
                                             start=(k == khi and t == 0),
                                             stop=(t == snt - 1))

                        # msg = p * hg in place, then num accumulation
                        hgv = hg[:, 0:snt, 0:RC * HC].rearrange(
                            "p t (r c h) -> p t r c h", h=H, c=C)
                        pb = p_sb[:, toff:toff + snt, :].rearrange(
                            "p t (r o h) -> p t r o h", h=H, o=1
                        ).to_broadcast([128, snt, RC, C, H])
                        mv = nc.vector.tensor_tensor(out=hgv, in0=hgv, in1=pb,
                                                     op=Alu.mult)
                        mv.ins.add_nosync_dependencies_from(
                            INOS([dve_w.ins.name]))
                        last = None
                        for t in range(snt):
                            for cb in range(3):
                                last = nc.tensor.matmul(
                                    out=nps[:, cb * 512:(cb + 1) * 512],
                                    lhsT=oh[:, t0 + t, :],
                                    rhs=hg[:, t, cb * 512:(cb + 1) * 512],
                                    start=False,
                                    stop=(k == khi and t == snt - 1))
                        ci = nc.tensor.sem_inc(csem, 1)
                        ci.ins.add_nosync_dependencies_from(
                            INOS([last.ins.name]))

                    # finalize: numn = num * (0.25/den), head-sum, + bias
                    dpair = finp.tile([DTW, 2, AC], f32, tag="dpair")
                    nc.scalar.copy(out=dpair[:],
                                   in_=zdn[0:DTW, DEN0:DEN0 + 2 * AC].rearrange(
                                       "d (e a) -> d e a", a=AC))
                    dsum = finp.tile([DTW, AC], f32, tag="dsum")
                    nc.vector.tensor_tensor(out=dsum[:], in0=dpair[:, 0, :],
                                            in1=dpair[:, 1, :], op=Alu.add)
                    invd = finp.tile([DTW, AC], f32, tag="invd")
                    nc.vector.reciprocal(out=invd[:], in_=dsum[:])
                    invdb = finp.tile([DTW, AC], bf16, tag="invdb")
                    nc.vector.tensor_scalar_mul(invdb[:], invd[:], 0.25)
                    nsb = finp.tile([DTW, RC, C, H], bf16, tag="nsb")
                    nc.scalar.copy(
                        out=nsb[:],
                        in_=nps[0:DTW, 0:RC * HC].rearrange(
                            "d (r c h) -> d r c h", h=H, c=C))
                    nc.vector.tensor_tensor(
                        out=nsb[:], in0=nsb[:],
                        in1=invdb[:].rearrange("d (r o h) -> d r o h",
                                               h=H, o=1
                                               ).to_broadcast([DTW, RC, C, H]),
                        op=Alu.mult)
                    hp = finp.tile([DTW, RC, C, 2], bf16, tag="hp")
                    nc.vector.tensor_tensor(out=hp[:],
                                            in0=nsb[:, :, :, 0:2],
                                            in1=nsb[:, :, :, 2:4], op=Alu.add)
                    ob = finp.tile([DTW, RC, C], f32, tag="ob")
                    nc.vector.tensor_tensor(out=ob[:], in0=hp[:, :, :, 0],
                                            in1=hp[:, :, :, 1], op=Alu.add)
                    nc.vector.tensor_tensor(out=ob[:], in0=ob[:],
                                            in1=bias_sl[:], op=Alu.add)
                    nc.sync.dma_start(
                        out_d[:, dt * DTW:(dt + 1) * DTW, :].rearrange(
                            "r d f -> d r f"), ob[:])

    nc.compile()
    return nc


# --------------------------------------------------------------------------
# public entry point
# --------------------------------------------------------------------------
def kernel(x, edge_index, W, att_src, att_dst, bias):
    key = hash(np.asarray(edge_index).tobytes())
    if key not in _CACHE:
        ed = _prep_edges(edge_index)
        _CACHE[key] = (_build_program(ed), ed)
    nc, ed = _CACHE[key]

    in_maps = _make_in_maps(x, W, att_src, att_dst, bias, ed)
    from concourse import bass_utils
    res = bass_utils.run_bass_kernel_spmd(nc, in_maps, core_ids=list(range(NCORES)))
    outs = [res.results[c]["out"] for c in range(NCORES)]
    out = np.concatenate(outs, axis=0).reshape(B, S, N, F).astype(np.float32)
    return out
